# revision 1
# baseline (speedup 1.0000x reference)
# Bass/Trainium2 kernel for nn_CNN_tcn (dense_cnn, 8-core data parallel).
#
# Math (per sample b; reference semantics):
#   s:[A,D,CIN] -> transpose -> x:[CIN,A,D]
#   3x dblock:  y=dilconv(x), LN over all elems, 1x1 conv, exact gelu
#   concat(x1,x2,x3) -> LN12 -> cw 1x1 -> c2w (1,D) conv -> LN16
#   concat w -> c3 1x1 -> LN over A -> out[A]
#
# Key algebraic folds used here:
#  * pw o LN(y):  z = (F(x) - mean(y)*rho)*rstd(y), F = pw∘dil fused kernel,
#    applied via gelu's scale/bias (ACT computes gelu(scale*in+bias)).
#  * cw+c2w fused into Kj[o,(c,d)] per block; LN12 affine handled analytically
#    (sigma12 cancels inside LN16 up to an eps rescale).
#  * LN16 + c3 + final-LN: per-sample additive constants drop under the final
#    LN's mean subtraction, so only rstd16*q + c3w*w is materialized.
#
# Layout: per sample 2 SBUF tiles [128, A]; partition row r=4*d+c (d-major,
# channel fast) which is exactly what the DMA xbar transpose of the native
# [A, D, CIN] input produces. Convs become [128,128] matmuls with
# host-precomputed banded weight matrices.

import sys

sys.path.insert(0, "/opt/trn_rl_repo")

import numpy as np
import ml_dtypes

import concourse.bacc as bacc
import concourse.tile as tile
import concourse.mybir as mybir
from concourse.bass_utils import run_bass_kernel_spmd

F32 = mybir.dt.float32
BF16 = mybir.dt.bfloat16
I32 = mybir.dt.int32
ALU = mybir.AluOpType
ACTF = mybir.ActivationFunctionType

B, A, D, CIN = 256, 512, 64, 4
C1, C2 = 4, 16
EPS = 1e-5
NCORES = 8
SPC = B // NCORES          # 32 samples per core
G = 8                      # samples per group
NG = SPC // G
NROW = CIN * D             # 256 rows = 2 ptiles
MAGIC = 0x5F3759DF


# ---------------------------------------------------------------- host consts
def _band_matrix(w3):
    """w3:[o,i,3] -> T[256,256]; T[4d'+i, 4d+o] = sum_t w3[o,i,t]*[d'==d+2t-2]."""
    T = np.zeros((NROW, NROW), np.float32)
    for t in range(3):
        delta = 2 * t - 2
        for d in range(D):
            dp = d + delta
            if 0 <= dp < D:
                # rows 4dp+i, cols 4d+o
                T[4 * dp : 4 * dp + 4, 4 * d : 4 * d + 4] += w3[:, :, t].T
    return T


def build_consts(inp):
    c = {}
    dil = [inp["d1w1"], inp["d2w1"], inp["d3w1"]]   # [4,4,1,3]
    pw = [inp["d1w2"], inp["d2w2"], inp["d3w2"]]    # [4,4,1,1]
    cw = inp["cw"][:, :, 0, 0]                       # [4,12]
    c2w = inp["c2w"][:, :, 0, :]                     # [16,4,64]
    c3 = inp["c3w"][0, :, 0, 0]                      # [17]

    TY = np.zeros((3, 2, 2, 128, 128), np.float32)
    TF = np.zeros((3, 2, 2, 128, 128), np.float32)
    RHO = np.zeros((3, 128, G), np.float32)
    for j in range(3):
        w3 = np.asarray(dil[j])[:, :, 0, :]          # [o,i,3]
        pj = np.asarray(pw[j])[:, :, 0, 0]           # [o,c]
        ty = _band_matrix(w3)
        fused = np.einsum("oc,cit->oit", pj, w3)
        tf = _band_matrix(fused)
        for kk in range(2):
            for mm in range(2):
                TY[j, kk, mm] = ty[128 * kk : 128 * kk + 128, 128 * mm : 128 * mm + 128]
                TF[j, kk, mm] = tf[128 * kk : 128 * kk + 128, 128 * mm : 128 * mm + 128]
        rho = pj.sum(axis=1)                          # [4]
        RHO[j] = np.tile(rho, 32)[:, None].repeat(G, axis=1)  # row p -> rho[p%4]
    c["ty"] = TY.astype(ml_dtypes.bfloat16)
    c["tf"] = TF.astype(ml_dtypes.bfloat16)
    c["rho"] = RHO.astype(np.float32)

    # c2-fused kernels: Kj[o, 4d+cc] = sum_m c2w[o,m,d]*cw[m, 4j+cc]
    KJ = np.zeros((3, 2, 128, 16), np.float32)
    beta = np.zeros(16, np.float32)
    for j in range(3):
        kj = np.einsum("omd,mc->ocd", np.asarray(c2w), np.asarray(cw)[:, 4 * j : 4 * j + 4])
        # rows r=4d+cc -> [256,16]
        kmat = np.zeros((NROW, 16), np.float32)
        for d in range(D):
            kmat[4 * d : 4 * d + 4, :] = kj[:, :, d].T
        beta += kmat.sum(axis=0)
        KJ[j, 0] = kmat[:128]
        KJ[j, 1] = kmat[128:]
    c["kj"] = KJ.astype(ml_dtypes.bfloat16)

    # beta pattern on c2-psum rows (row 32s'+o, o<16): beta1[p] = beta[p%32] if p%32<16
    b1 = np.zeros((128, 1), np.float32)
    for p in range(128):
        if p % 32 < 16:
            b1[p, 0] = beta[p % 32]
    c["beta1"] = b1
    c["meanbeta"] = float(beta.sum() / 16.0)
    c["ebeta2"] = float((beta**2).sum() / 16.0)

    # blockdiag c3 for q-MM: col 32s'(+16 for bank B) has c3[:16] at rows 32s'..+16
    c3bdA = np.zeros((128, 128), np.float32)
    c3bdB = np.zeros((128, 128), np.float32)
    for sp in range(4):
        c3bdA[32 * sp : 32 * sp + 16, 32 * sp] = np.asarray(c3)[:16]
        c3bdB[32 * sp : 32 * sp + 16, 32 * sp + 16] = np.asarray(c3)[:16]
    c["c3bda"] = c3bdA
    c["c3bdb"] = c3bdB
    wbdA = np.zeros((8, 128), np.float32)
    wbdB = np.zeros((8, 128), np.float32)
    for sp in range(4):
        wbdA[sp, 32 * sp] = float(np.asarray(c3)[16])
        wbdB[4 + sp, 32 * sp + 16] = float(np.asarray(c3)[16])
    c["wbda"] = wbdA
    c["wbdb"] = wbdB

    # blockdiag-16 ones for LN16 combine: col r = ones over rows [32*(r//32), +16)
    bd16 = np.zeros((128, 128), np.float32)
    for r in range(128):
        blk = r // 32
        bd16[32 * blk : 32 * blk + 16, r] = 1.0
    c["bd16"] = bd16
    c["ones"] = np.ones((128, 128), np.float32)
    return c


# ---------------------------------------------------------------- device code
def build_program():
    nc = bacc.Bacc("TRN2", target_bir_lowering=False, debug=False,
                   num_devices=NCORES)

    s_dram = nc.dram_tensor("s", [SPC, A, D, CIN], F32, kind="ExternalInput")
    w_dram = nc.dram_tensor("w", [SPC, A], F32, kind="ExternalInput")
    ty_dram = nc.dram_tensor("ty", [3, 2, 2, 128, 128], BF16, kind="ExternalInput")
    tf_dram = nc.dram_tensor("tf", [3, 2, 2, 128, 128], BF16, kind="ExternalInput")
    kj_dram = nc.dram_tensor("kj", [3, 2, 128, 16], BF16, kind="ExternalInput")
    rho_dram = nc.dram_tensor("rho", [3, 128, G], F32, kind="ExternalInput")
    beta1_dram = nc.dram_tensor("beta1", [128, 1], F32, kind="ExternalInput")
    c3bda_dram = nc.dram_tensor("c3bda", [128, 128], F32, kind="ExternalInput")
    c3bdb_dram = nc.dram_tensor("c3bdb", [128, 128], F32, kind="ExternalInput")
    wbda_dram = nc.dram_tensor("wbda", [8, 128], F32, kind="ExternalInput")
    wbdb_dram = nc.dram_tensor("wbdb", [8, 128], F32, kind="ExternalInput")
    bd16_dram = nc.dram_tensor("bd16", [128, 128], F32, kind="ExternalInput")
    ones_dram = nc.dram_tensor("ones", [128, 128], F32, kind="ExternalInput")
    sc_dram = nc.dram_tensor("sc", [128, 2], F32, kind="ExternalInput")  # meanbeta, ebeta2 bcast
    out_dram = nc.dram_tensor("out", [SPC, A], F32, kind="ExternalOutput")

    with tile.TileContext(nc) as tc:
        with (
            tc.tile_pool(name="consts", bufs=1) as consts,
            tc.tile_pool(name="dram", bufs=1, space="DRAM") as drampool,
            tc.tile_pool(name="data", bufs=12) as data,       # in1/in2/in3 tiles
            tc.tile_pool(name="xdata", bufs=12) as xdata,      # x1/x2/x3 tiles
            tc.tile_pool(name="stats", bufs=6) as stats,
            tc.tile_pool(name="tiny", bufs=8) as tiny,
            tc.tile_pool(name="grp", bufs=3) as grp,
            tc.tile_pool(name="outp", bufs=4) as outp,
            tc.tile_pool(name="psum", bufs=2, space="PSUM") as cpsum,
            tc.tile_pool(name="spsum", bufs=1, space="PSUM") as spsum,
            tc.tile_pool(name="c2psum", bufs=2, space="PSUM") as c2psum,
        ):
            # ---- load consts to SBUF
            ty_sb = consts.tile([128, 3, 2, 2, 128], BF16, tag="cty")
            tf_sb = consts.tile([128, 3, 2, 2, 128], BF16, tag="ctf")
            kj_sb = consts.tile([128, 3, 2, 16], BF16, tag="ckj")
            rho_sb = consts.tile([128, 3, G], F32, tag="crho")
            beta1_sb = consts.tile([128, 1], F32, tag="cbeta")
            c3bda_sb = consts.tile([128, 128], F32, tag="cc3a")
            c3bdb_sb = consts.tile([128, 128], F32, tag="cc3b")
            wbda_sb = consts.tile([8, 128], F32, tag="cwa")
            wbdb_sb = consts.tile([8, 128], F32, tag="cwb")
            bd16_sb = consts.tile([128, 128], F32, tag="cbd")
            ones_sb = consts.tile([128, 128], F32, tag="cones")
            sc_sb = consts.tile([128, 2], F32, tag="csc")
            magic_sb = consts.tile([128, 2 * G], I32, tag="cmagic")
            eps_sb = consts.tile([128, 1], F32, tag="ceps")
            # ty/tf need per-[128,128] partition-major loads: DMA each chunk
            for j in range(3):
                for kk in range(2):
                    for mm in range(2):
                        nc.sync.dma_start(ty_sb[:, j, kk, mm, :], ty_dram[j, kk, mm])
                        nc.sync.dma_start(tf_sb[:, j, kk, mm, :], tf_dram[j, kk, mm])
                for kk in range(2):
                    nc.sync.dma_start(kj_sb[:, j, kk, :], kj_dram[j, kk])
                nc.sync.dma_start(rho_sb[:, j, :], rho_dram[j])
            nc.sync.dma_start(beta1_sb[:], beta1_dram[:])
            nc.sync.dma_start(c3bda_sb[:], c3bda_dram[:])
            nc.sync.dma_start(c3bdb_sb[:], c3bdb_dram[:])
            nc.sync.dma_start(wbda_sb[:], wbda_dram[:])
            nc.sync.dma_start(wbdb_sb[:], wbdb_dram[:])
            nc.sync.dma_start(bd16_sb[:], bd16_dram[:])
            nc.sync.dma_start(ones_sb[:], ones_dram[:])
            nc.sync.dma_start(sc_sb[:], sc_dram[:])
            nc.vector.memset(magic_sb[:], MAGIC)
            nc.vector.memset(eps_sb[:], EPS)

            def rsqrt(v_ap, k, tagp):
                """v_ap: [128,k] f32 SBUF (contiguous), >0. Returns [128,k] f32 tile."""
                sh = tiny.tile([128, k], I32, tag=f"rs_sh{tagp}")
                nc.vector.tensor_scalar(
                    out=sh[:], in0=v_ap.bitcast(I32), scalar1=1, scalar2=None,
                    op0=ALU.logical_shift_right)
                x = tiny.tile([128, k], F32, tag=f"rs_x{tagp}")
                nc.vector.tensor_tensor(
                    out=x[:].bitcast(I32), in0=magic_sb[:, :k], in1=sh[:],
                    op=ALU.subtract)
                xx = tiny.tile([128, k], F32, tag=f"rs_xx{tagp}")
                t3 = tiny.tile([128, k], F32, tag=f"rs_t3{tagp}")
                for _ in range(2):
                    nc.vector.tensor_tensor(out=xx[:], in0=x[:], in1=x[:], op=ALU.mult)
                    nc.vector.tensor_tensor(out=xx[:], in0=xx[:], in1=v_ap, op=ALU.mult)
                    nc.vector.tensor_scalar(out=t3[:], in0=xx[:], scalar1=-0.5,
                                            scalar2=1.5, op0=ALU.mult, op1=ALU.add)
                    nc.vector.tensor_tensor(out=x[:], in0=x[:], in1=t3[:], op=ALU.mult)
                return x

            # stage DRAM scratch, bf16 [SPC, A, 256]
            stage = drampool.tile([SPC, A, NROW], BF16, tag="stage")
            s_flat = s_dram[:].rearrange("s a d c -> s a (d c)")
            for s in range(G):
                nc.gpsimd.dma_start(stage[s], s_flat[s])  # group 0 prefetch

            for g in range(NG):
                smp = [g * G + s for s in range(G)]
                # ---- transpose loads: in1[s][pt] [128, A] bf16
                in1 = [[data.tile([128, A], BF16, tag=f"in1_{pt}", name=f"in1_{s}_{pt}")
                        for pt in range(2)] for s in range(G)]
                for s in range(G):
                    for pt in range(2):
                        nc.sync.dma_start(
                            in1[s][pt][:],
                            stage[smp[s], :, 128 * pt : 128 * pt + 128],
                            transpose=True)

                cur = in1
                xs = []
                st12 = [stats.tile([128, 6, 6], F32, tag="st12", name=f"st12_{s}",
                                   bufs=2 * G) for s in range(G)]
                for j in range(3):
                    gyh = [grp.tile([128, G], F32, tag=f"gy{h}", name=f"gy_{h}")
                           for h in range(2)]
                    for s in range(G):
                        # y conv (stats only) and fused z conv
                        yps = [cpsum.tile([128, A], F32, tag="ypsum", name=f"yps{s}_{mm}") for mm in range(2)]
                        for mm in range(2):
                            for kk in range(2):
                                nc.tensor.matmul(
                                    yps[mm][:], ty_sb[:, j, kk, mm, :], cur[s][kk][:],
                                    start=(kk == 0), stop=(kk == 1))
                        st6 = stats.tile([128, 2, 6], F32, tag="st6")
                        for mm in range(2):
                            nc.vector.bn_stats(out=st6[:, mm, :], in_=yps[mm][:])
                        mv = stats.tile([128, 2], F32, tag="mv")
                        nc.vector.bn_aggr(out=mv[:], in_=st6[:])
                        hh, sl = s // 4, s % 4
                        nc.vector.tensor_copy(gyh[hh][:, 2 * sl : 2 * sl + 1],
                                              mv[:, 0:1])
                        mm2 = stats.tile([128, 1], F32, tag="mm2")
                        nc.vector.tensor_tensor(out=mm2[:], in0=mv[:, 0:1],
                                                in1=mv[:, 0:1], op=ALU.mult)
                        nc.vector.tensor_tensor(
                            out=gyh[hh][:, 2 * sl + 1 : 2 * sl + 2],
                            in0=mm2[:], in1=mv[:, 1:2], op=ALU.add)
                    # batched stats -> gelu scale/bias, per half-group of 4
                    rstdYh, biasYh = [], []
                    for h in range(2):
                        bc = spsum.tile([128, G], F32, tag="small", name=f"bc{h}")
                        nc.tensor.matmul(bc[:], ones_sb[:], gyh[h][:],
                                         start=True, stop=True)
                        meanY = tiny.tile([128, 4], F32, tag=f"meanY{h}")
                        nc.vector.tensor_scalar_mul(meanY[:], bc[:, 0:G:2], 1.0 / 128)
                        mm2b = tiny.tile([128, 4], F32, tag=f"mm2b{h}")
                        nc.vector.tensor_tensor(out=mm2b[:], in0=meanY[:],
                                                in1=meanY[:], op=ALU.mult)
                        varY = tiny.tile([128, 4], F32, tag=f"varY{h}")
                        nc.vector.scalar_tensor_tensor(
                            out=varY[:], in0=bc[:, 1:G:2], scalar=1.0 / 128,
                            in1=mm2b[:], op0=ALU.mult, op1=ALU.subtract)
                        nc.vector.tensor_scalar_add(varY[:], varY[:], EPS)
                        rstdY = rsqrt(varY[:], 4, f"y{h}")
                        biasY = tiny.tile([128, 4], F32, tag=f"biasY{h}")
                        nc.vector.tensor_tensor(out=biasY[:], in0=meanY[:],
                                                in1=rstdY[:], op=ALU.mult)
                        nc.vector.scalar_tensor_tensor(
                            out=biasY[:], in0=biasY[:], scalar=-1.0,
                            in1=rho_sb[:, j, 0:4], op0=ALU.mult, op1=ALU.mult)
                        rstdYh.append(rstdY)
                        biasYh.append(biasY)
                    xj = [[xdata.tile([128, A], BF16, tag=f"x{j}_{pt}", name=f"x{j}_{s}_{pt}")
                           for pt in range(2)] for s in range(G)]
                    for s in range(G):
                        zps = [cpsum.tile([128, A], F32, tag="zpsum", name=f"zps{s}_{mm}", bufs=3) for mm in range(2)]
                        for mm in range(2):
                            for kk in range(2):
                                nc.tensor.matmul(
                                    zps[mm][:], tf_sb[:, j, kk, mm, :], cur[s][kk][:],
                                    start=(kk == 0), stop=(kk == 1))
                        for pt in range(2):
                            hh, sl = s // 4, s % 4
                            nc.scalar.activation(
                                out=xj[s][pt][:], in_=zps[pt][:], func=ACTF.Gelu,
                                bias=biasYh[hh][:, sl : sl + 1],
                                scale=rstdYh[hh][:, sl : sl + 1])
                            nc.vector.bn_stats(out=st12[s][:, 2 * j + pt, :],
                                               in_=xj[s][pt][:])
                    xs.append(xj)
                    if j == 0 and g + 1 < NG:
                        for s2 in range(G):
                            sg = (g + 1) * G + s2
                            nc.gpsimd.dma_start(stage[sg], s_flat[sg])
                    if j < 2:
                        nxt = [[data.tile([128, A], BF16, tag=f"in{j+2}_{pt}", name=f"in{j+2}_{s}_{pt}")
                                for pt in range(2)] for s in range(G)]
                        for s in range(G):
                            for pt in range(2):
                                nc.gpsimd.tensor_tensor(
                                    out=nxt[s][pt][:], in0=cur[s][pt][:],
                                    in1=xj[s][pt][:], op=ALU.add)
                        cur = nxt

                # ---- LN12 stats: x1/x2 raw sums + x3 bn stats, all per partition
                # g12 cols: 2s = total sum, 2s+1 = total sumsq (per partition,
                # over 3072 elems); ones-MM then yields grand totals.
                g12 = grp.tile([128, 2 * G], F32, tag="g12")
                for s in range(G):
                    mv12 = stats.tile([128, 2], F32, tag="mv12")
                    nc.vector.bn_aggr(out=mv12[:], in_=st12[s][:])
                    nc.vector.tensor_copy(g12[:, 2 * s : 2 * s + 1], mv12[:, 0:1])
                    mm2c = stats.tile([128, 1], F32, tag="mm2c")
                    nc.vector.tensor_tensor(out=mm2c[:], in0=mv12[:, 0:1],
                                            in1=mv12[:, 0:1], op=ALU.mult)
                    nc.vector.tensor_tensor(out=g12[:, 2 * s + 1 : 2 * s + 2],
                                            in0=mm2c[:], in1=mv12[:, 1:2], op=ALU.add)
                bc12 = spsum.tile([128, 2 * G], F32, tag="small")
                nc.tensor.matmul(bc12[:], ones_sb[:], g12[:], start=True, stop=True)
                m12 = tiny.tile([128, G], F32, tag="m12")
                nc.vector.tensor_scalar_mul(m12[:], bc12[:, 0 : 2 * G : 2], 1.0 / 128)
                mm2d = tiny.tile([128, G], F32, tag="mm2d")
                nc.vector.tensor_tensor(out=mm2d[:], in0=m12[:], in1=m12[:], op=ALU.mult)
                s12 = tiny.tile([128, G], F32, tag="s12")  # sigma12^2 = var12+eps
                nc.vector.scalar_tensor_tensor(
                    out=s12[:], in0=bc12[:, 1 : 2 * G : 2], scalar=1.0 / 128,
                    in1=mm2d[:], op0=ALU.mult, op1=ALU.subtract)
                nc.vector.tensor_scalar_add(s12[:], s12[:], EPS)
                # row-aligned [128,2] (colA=bankA rows 32s', colB=bankB)
                m12r = tiny.tile([128, 2], F32, tag="m12r")
                s12r = tiny.tile([128, 2], F32, tag="s12r")
                for sp in range(4):
                    rr = slice(32 * sp, 32 * sp + 16)
                    nc.vector.tensor_copy(m12r[rr, 0:1], m12[rr, sp : sp + 1])
                    nc.vector.tensor_copy(m12r[rr, 1:2], m12[rr, 4 + sp : 5 + sp])
                    nc.vector.tensor_copy(s12r[rr, 0:1], s12[rr, sp : sp + 1])
                    nc.vector.tensor_copy(s12r[rr, 1:2], s12[rr, 4 + sp : 5 + sp])

                # ---- c2 conv, bank A = samples 0..3 (rows 32s'), bank B = 4..7
                c2ps = [c2psum.tile([128, A], F32, tag="c2ps", name=f"c2ps{bb}") for bb in range(2)]
                for j in range(3):
                    for kk in range(2):
                        for bank in range(2):
                            for sp in range(4):
                                s = 4 * bank + sp
                                nc.tensor.matmul(
                                    c2ps[bank][32 * sp : 32 * sp + 16, :],
                                    kj_sb[:, j, kk, :], xs[j][s][kk][:],
                                    start=(j == 0 and kk == 0),
                                    stop=(j == 2 and kk == 1),
                                    tile_position=(0, 32 * sp))
                                first = False
                # LN16 combine inputs: per bank rhs [128,3]: mean, var+mean^2, beta*mean
                bcs = spsum.tile([128, 6], F32, tag="small")
                for bank in range(2):
                    mvc = stats.tile([128, 2], F32, tag="mvc")
                    stc = stats.tile([128, 1, 6], F32, tag="stc")
                    nc.vector.bn_stats(out=stc[:, 0, :], in_=c2ps[bank][:])
                    nc.vector.bn_aggr(out=mvc[:], in_=stc[:])
                    rhsc = stats.tile([128, 3], F32, tag="rhsc")
                    nc.vector.tensor_copy(rhsc[:, 0:1], mvc[:, 0:1])
                    mm2e = stats.tile([128, 1], F32, tag="mm2e")
                    nc.vector.tensor_tensor(out=mm2e[:], in0=mvc[:, 0:1],
                                            in1=mvc[:, 0:1], op=ALU.mult)
                    nc.vector.tensor_tensor(out=rhsc[:, 1:2], in0=mm2e[:],
                                            in1=mvc[:, 1:2], op=ALU.add)
                    nc.vector.tensor_tensor(out=rhsc[:, 2:3], in0=mvc[:, 0:1],
                                            in1=beta1_sb[:], op=ALU.mult)
                    nc.tensor.matmul(bcs[:, 3 * bank : 3 * bank + 3], bd16_sb[:],
                                     rhsc[:], start=True, stop=True)
                # var16 pipeline on [128,2] (col = bank); S0,S1,S2 at strided cols
                ex = tiny.tile([128, 2], F32, tag="ex")
                nc.vector.tensor_scalar_mul(ex[:], bcs[:, 0:6:3], 1.0 / 16)
                ex2 = tiny.tile([128, 2], F32, tag="ex2")
                nc.vector.tensor_scalar_mul(ex2[:], bcs[:, 1:6:3], 1.0 / 16)
                exa = tiny.tile([128, 2], F32, tag="exa")
                nc.vector.tensor_tensor(out=exa[:], in0=m12r[:], in1=bcs[:, 2:6:3],
                                        op=ALU.mult)
                nc.vector.tensor_scalar_mul(exa[:], exa[:], 1.0 / 16)
                ea = tiny.tile([128, 2], F32, tag="ea")
                nc.vector.tensor_scalar(out=ea[:], in0=m12r[:],
                                        scalar1=sc_sb[:, 0:1], scalar2=None,
                                        op0=ALU.mult)
                ea2 = tiny.tile([128, 2], F32, tag="ea2")
                nc.vector.tensor_tensor(out=ea2[:], in0=m12r[:], in1=m12r[:],
                                        op=ALU.mult)
                nc.vector.tensor_scalar(out=ea2[:], in0=ea2[:],
                                        scalar1=sc_sb[:, 1:2], scalar2=None,
                                        op0=ALU.mult)
                ctr = tiny.tile([128, 2], F32, tag="ctr")
                nc.vector.tensor_tensor(out=ctr[:], in0=ex[:], in1=ea[:],
                                        op=ALU.subtract)
                v16 = tiny.tile([128, 2], F32, tag="v16")
                nc.vector.scalar_tensor_tensor(out=v16[:], in0=exa[:], scalar=-2.0,
                                               in1=ex2[:], op0=ALU.mult, op1=ALU.add)
                nc.vector.tensor_tensor(out=v16[:], in0=v16[:], in1=ea2[:], op=ALU.add)
                nc.vector.tensor_tensor(out=ctr[:], in0=ctr[:], in1=ctr[:], op=ALU.mult)
                nc.vector.tensor_tensor(out=v16[:], in0=v16[:], in1=ctr[:],
                                        op=ALU.subtract)
                nc.vector.scalar_tensor_tensor(out=v16[:], in0=s12r[:], scalar=EPS,
                                               in1=v16[:], op0=ALU.mult, op1=ALU.add)
                rstd16 = rsqrt(v16[:], 2, "c")

                # ---- scaled copy, q projection, final LN
                wtile = grp.tile([8, A], F32, tag="wtile")
                nc.sync.dma_start(wtile[:], w_dram[g * G : g * G + 8, :])
                q2 = stats.tile([128, 4], F32, tag="q2")
                outsb = [outp.tile([128, A], F32, tag="outsb", name=f"outsb{bb}") for bb in range(2)]
                for bank in range(2):
                    c2sb = grp.tile([128, A], F32, tag="c2sb")
                    nc.scalar.activation(out=c2sb[:], in_=c2ps[bank][:],
                                         func=ACTF.Copy,
                                         scale=rstd16[:, bank : bank + 1])
                    qps = spsum.tile([128, A], F32, tag="small")
                    nc.tensor.matmul(qps[:], c3bda_sb[:] if bank == 0 else c3bdb_sb[:],
                                     c2sb[:], start=True, stop=False)
                    nc.tensor.matmul(qps[:], wbda_sb[:] if bank == 0 else wbdb_sb[:],
                                     wtile[:], start=False, stop=True)
                    stq = stats.tile([128, 1, 6], F32, tag="stq")
                    nc.vector.bn_stats(out=stq[:, 0, :], in_=qps[:])
                    nc.vector.bn_aggr(out=q2[:, 2 * bank : 2 * bank + 2], in_=stq[:])
                    va = tiny.tile([128, 1], F32, tag="va")
                    nc.vector.tensor_scalar_add(va[:], q2[:, 2 * bank + 1 : 2 * bank + 2], EPS)
                    rstda = rsqrt(va[:], 1, "a")
                    nc.vector.tensor_scalar(
                        out=outsb[bank][:], in0=qps[:],
                        scalar1=q2[:, 2 * bank : 2 * bank + 1], scalar2=rstda[:, 0:1],
                        op0=ALU.subtract, op1=ALU.mult)
                # valid rows: bank A at 32*sp, bank B at 32*sp+16 -> out[g*8+...]
                for bank in range(2):
                    src = outsb[bank][:].rearrange("(sp u) a -> sp u a", u=32)
                    nc.sync.dma_start(
                        out_dram[g * G + 4 * bank : g * G + 4 * bank + 4, :],
                        src[:, 16 * bank, :])
    nc.compile()
    return nc


_CACHE = {}


def kernel(**inputs):
    inputs = {k: np.asarray(v) for k, v in inputs.items()}
    consts = build_consts(inputs)
    if "nc" not in _CACHE:
        _CACHE["nc"] = build_program()
    nc = _CACHE["nc"]

    sc = np.zeros((128, 2), np.float32)
    sc[:, 0] = consts["meanbeta"]
    sc[:, 1] = consts["ebeta2"]
    base = {
        "ty": np.ascontiguousarray(consts["ty"]),
        "tf": np.ascontiguousarray(consts["tf"]),
        "kj": np.ascontiguousarray(consts["kj"]),
        "rho": np.ascontiguousarray(consts["rho"]),
        "beta1": np.ascontiguousarray(consts["beta1"]),
        "c3bda": np.ascontiguousarray(consts["c3bda"]),
        "c3bdb": np.ascontiguousarray(consts["c3bdb"]),
        "wbda": np.ascontiguousarray(consts["wbda"]),
        "wbdb": np.ascontiguousarray(consts["wbdb"]),
        "bd16": np.ascontiguousarray(consts["bd16"]),
        "ones": np.ascontiguousarray(consts["ones"]),
        "sc": sc,
    }
    in_maps = []
    for c in range(NCORES):
        m = dict(base)
        m["s"] = np.ascontiguousarray(inputs["s"][c * SPC : (c + 1) * SPC])
        m["w"] = np.ascontiguousarray(inputs["w"][c * SPC : (c + 1) * SPC])
        in_maps.append(m)
    _CACHE["in_maps"] = in_maps
    res = run_bass_kernel_spmd(nc, in_maps, core_ids=list(range(NCORES)))
    out = np.concatenate([r["out"] for r in res.results], axis=0)
    return out.astype(np.float32)



# revision 32
# speedup vs baseline: 1.0112x; 1.0112x over previous
# Bass/Trainium2 kernel for nn_CNN_tcn (dense_cnn, 8-core data parallel).
#
# Math (per sample b; reference semantics):
#   s:[A,D,CIN] -> transpose -> x:[CIN,A,D]
#   3x dblock:  y=dilconv(x), LN over all elems, 1x1 conv, exact gelu
#   concat(x1,x2,x3) -> LN12 -> cw 1x1 -> c2w (1,D) conv -> LN16
#   concat w -> c3 1x1 -> LN over A -> out[A]
#
# Key algebraic folds used here:
#  * pw o LN(y):  z = (F(x) - mean(y)*rho)*rstd(y), F = pw∘dil fused kernel,
#    applied via gelu's scale/bias (ACT computes gelu(scale*in+bias)).
#  * cw+c2w fused into Kj[o,(c,d)] per block; LN12 affine handled analytically
#    (sigma12 cancels inside LN16 up to an eps rescale; that eps*sigma12^2
#    correction is ~1e-5 relative and is replaced by a fixed eps floor, so
#    LN12's variance is never computed).  LN12's mean comes for free from the
#    gelu accumulator outputs.
#  * LN16 + c3 + final-LN: per-sample additive constants drop under the final
#    LN's mean subtraction, so only rstd16*q + c3w*w is materialized.
#
# Layout: per sample one SBUF pair tile [128, 2*A] bf16; cols [0,A) hold
# partition-rows r=4d+c for d<32 (ptile 0), cols [A,2A) ptile 1.  Convs are
# [128,128] matmuls with host-precomputed banded weight matrices.

import sys

sys.path.insert(0, "/opt/trn_rl_repo")

import numpy as np
import ml_dtypes

import concourse.bacc as bacc
import concourse.tile as tile
import concourse.mybir as mybir
from concourse.bass_utils import run_bass_kernel_spmd

F32 = mybir.dt.float32
BF16 = mybir.dt.bfloat16
I32 = mybir.dt.int32
ALU = mybir.AluOpType
ACTF = mybir.ActivationFunctionType

B, A, D, CIN = 256, 512, 64, 4
C1, C2 = 4, 16
EPS = 1e-5
NCORES = 8
SPC = B // NCORES          # 32 samples per core
G = 8                      # samples per group
NG = SPC // G
NROW = CIN * D             # 256 rows = 2 ptiles
MAGIC = 0x5F3759DF
N12 = 3 * NROW * A         # elems per sample in concat(x1,x2,x3)


# ---------------------------------------------------------------- host consts
def _band_matrix(w3):
    """w3:[o,i,3] -> T[256,256]; T[4d'+i, 4d+o] = sum_t w3[o,i,t]*[d'==d+2t-2]."""
    T = np.zeros((NROW, NROW), np.float32)
    for t in range(3):
        delta = 2 * t - 2
        for d in range(D):
            dp = d + delta
            if 0 <= dp < D:
                T[4 * dp : 4 * dp + 4, 4 * d : 4 * d + 4] += w3[:, :, t].T
    return T


def build_consts(inp):
    c = {}
    dil = [inp["d1w1"], inp["d2w1"], inp["d3w1"]]   # [4,4,1,3]
    pw = [inp["d1w2"], inp["d2w2"], inp["d3w2"]]    # [4,4,1,1]
    cw = inp["cw"][:, :, 0, 0]                       # [4,12]
    c2w = inp["c2w"][:, :, 0, :]                     # [16,4,64]
    c3 = inp["c3w"][0, :, 0, 0]                      # [17]

    TY = np.zeros((3, 2, 2, 128, 128), np.float32)
    TF = np.zeros((3, 2, 2, 128, 128), np.float32)
    RHO = np.zeros((3, 128, 4), np.float32)
    for j in range(3):
        w3 = np.asarray(dil[j])[:, :, 0, :]          # [o,i,3]
        pj = np.asarray(pw[j])[:, :, 0, 0]           # [o,c]
        ty = _band_matrix(w3)
        fused = np.einsum("oc,cit->oit", pj, w3)
        tf = _band_matrix(fused)
        for kk in range(2):
            for mm in range(2):
                TY[j, kk, mm] = ty[128 * kk : 128 * kk + 128, 128 * mm : 128 * mm + 128]
                TF[j, kk, mm] = tf[128 * kk : 128 * kk + 128, 128 * mm : 128 * mm + 128]
        rho = pj.sum(axis=1)                          # [4]
        RHO[j] = np.tile(rho, 32)[:, None].repeat(4, axis=1)  # row p -> rho[p%4]
    c["ty"] = TY.astype(ml_dtypes.bfloat16)
    c["tf"] = TF.astype(ml_dtypes.bfloat16)
    c["rho"] = RHO.astype(np.float32)

    # c2-fused kernels: Kj[o, 4d+cc] = sum_m c2w[o,m,d]*cw[m, 4j+cc]
    # col 16 = ones (accumulates per-sample column sums of concat(x) for the
    # LN12 mean); cols 17..31 = 0 (zero-fill the unused psum rows).
    KJ = np.zeros((3, 2, 128, 32), np.float32)
    KJ[:, :, :, 16] = 1.0
    beta = np.zeros(16, np.float32)
    for j in range(3):
        kj = np.einsum("omd,mc->ocd", np.asarray(c2w), np.asarray(cw)[:, 4 * j : 4 * j + 4])
        kmat = np.zeros((NROW, 16), np.float32)
        for d in range(D):
            kmat[4 * d : 4 * d + 4, :] = kj[:, :, d].T
        beta += kmat.sum(axis=0)
        KJ[j, 0, :, :16] = kmat[:128]
        KJ[j, 1, :, :16] = kmat[128:]
    c["kj"] = KJ.astype(ml_dtypes.bfloat16)

    # beta pattern on c2-psum rows (row 32s'+o, o<16): beta1[p] = beta[p%32] if p%32<16
    b1 = np.zeros((128, 1), np.float32)
    for p in range(128):
        if p % 32 < 16:
            b1[p, 0] = beta[p % 32]
    c["beta1"] = b1
    c["meanbeta"] = float(beta.sum() / 16.0)
    c["ebeta2"] = float((beta**2).sum() / 16.0)

    # blockdiag c3 for q-MM: col 32s'(+16 for bank B) has c3[:16] at rows 32s'..+16
    c3bdA = np.zeros((128, 128), np.float32)
    c3bdB = np.zeros((128, 128), np.float32)
    for sp in range(4):
        c3bdA[32 * sp : 32 * sp + 16, 32 * sp] = np.asarray(c3)[:16]
        c3bdB[32 * sp : 32 * sp + 16, 32 * sp + 16] = np.asarray(c3)[:16]
    c["c3bda"] = c3bdA.astype(ml_dtypes.bfloat16)
    c["c3bdb"] = c3bdB.astype(ml_dtypes.bfloat16)
    wbdA = np.zeros((8, 128), np.float32)
    wbdB = np.zeros((8, 128), np.float32)
    for sp in range(4):
        wbdA[sp, 32 * sp] = float(np.asarray(c3)[16])
        wbdB[4 + sp, 32 * sp + 16] = float(np.asarray(c3)[16])
    c["wbda"] = wbdA.astype(ml_dtypes.bfloat16)
    c["wbdb"] = wbdB.astype(ml_dtypes.bfloat16)

    # blockdiag-16 ones for LN16 combine: col r = ones over rows [32*(r//32), +16)
    bd16 = np.zeros((128, 128), np.float32)
    for r in range(128):
        blk = r // 32
        bd16[32 * blk : 32 * blk + 16, r] = 1.0
    c["bd16"] = bd16
    c["ones"] = np.ones((128, 128), np.float32)
    return c


# ---------------------------------------------------------------- device code
def build_program():
    nc = bacc.Bacc("TRN2", target_bir_lowering=False, debug=False,
                   num_devices=NCORES)

    s_dram = nc.dram_tensor("s", [SPC, A, D, CIN], F32, kind="ExternalInput")
    w_dram = nc.dram_tensor("w", [SPC, A], F32, kind="ExternalInput")
    ty_dram = nc.dram_tensor("ty", [3, 2, 2, 128, 128], BF16, kind="ExternalInput")
    tf_dram = nc.dram_tensor("tf", [3, 2, 2, 128, 128], BF16, kind="ExternalInput")
    kj_dram = nc.dram_tensor("kj", [3, 2, 128, 32], BF16, kind="ExternalInput")
    rho_dram = nc.dram_tensor("rho", [3, 128, 4], F32, kind="ExternalInput")
    beta1_dram = nc.dram_tensor("beta1", [128, 1], F32, kind="ExternalInput")
    c3bda_dram = nc.dram_tensor("c3bda", [128, 128], BF16, kind="ExternalInput")
    c3bdb_dram = nc.dram_tensor("c3bdb", [128, 128], BF16, kind="ExternalInput")
    wbda_dram = nc.dram_tensor("wbda", [8, 128], BF16, kind="ExternalInput")
    wbdb_dram = nc.dram_tensor("wbdb", [8, 128], BF16, kind="ExternalInput")
    bd16_dram = nc.dram_tensor("bd16", [128, 128], F32, kind="ExternalInput")
    ones_dram = nc.dram_tensor("ones", [128, 128], F32, kind="ExternalInput")
    sc_dram = nc.dram_tensor("sc", [128, 2], F32, kind="ExternalInput")  # meanbeta, ebeta2 bcast
    out_dram = nc.dram_tensor("out", [SPC, A], F32, kind="ExternalOutput")

    with tile.TileContext(nc) as tc:
        with (
            tc.tile_pool(name="consts", bufs=1) as consts,
            tc.tile_pool(name="dram", bufs=1, space="DRAM") as drampool,
            tc.tile_pool(name="data", bufs=16) as data,        # input pair tiles
            tc.tile_pool(name="xdata", bufs=8) as xdata,       # x1/x2/x3 pair tiles
            tc.tile_pool(name="stats", bufs=6) as stats,
            tc.tile_pool(name="tiny", bufs=8) as tiny,
            tc.tile_pool(name="grp", bufs=3) as grp,
            tc.tile_pool(name="outp", bufs=4) as outp,
            tc.tile_pool(name="ypsum", bufs=2, space="PSUM") as ypsum,    # [128,512] x2
            tc.tile_pool(name="zpsum", bufs=3, space="PSUM") as zpsum,    # [128,512] x3
            tc.tile_pool(name="c2psum", bufs=2, space="PSUM") as c2psum,  # [128,512] x2
            tc.tile_pool(name="spsum", bufs=1, space="PSUM") as spsum,    # [128,512] x1
        ):
            # ---- load consts to SBUF
            ty_sb = consts.tile([128, 3, 2, 2, 128], BF16, tag="cty")
            tf_sb = consts.tile([128, 3, 2, 2, 128], BF16, tag="ctf")
            kj_sb = consts.tile([128, 3, 2, 32], BF16, tag="ckj")
            rho_sb = consts.tile([128, 3, 4], F32, tag="crho")
            beta1_sb = consts.tile([128, 1], F32, tag="cbeta")
            c3bda_sb = consts.tile([128, 128], BF16, tag="cc3a")
            c3bdb_sb = consts.tile([128, 128], BF16, tag="cc3b")
            wbda_sb = consts.tile([8, 128], BF16, tag="cwa")
            wbdb_sb = consts.tile([8, 128], BF16, tag="cwb")
            bd16_sb = consts.tile([128, 128], F32, tag="cbd")
            ones_sb = consts.tile([128, 128], F32, tag="cones")
            sc_sb = consts.tile([128, 2], F32, tag="csc")
            magic_sb = consts.tile([128, 4], I32, tag="cmagic")
            # everything input-related lives on the sync (HWDGE) ring: conv
            # weights, then per-sample stage (f32->bf16 converting copy to a
            # DRAM scratch) + xbar-transpose loads; the small group-end consts
            # follow the first group's loads
            stage = drampool.tile([SPC, A, NROW], BF16, tag="stage")
            s_flat = s_dram[:].rearrange("s a d c -> s a (d c)")
            nc.sync.dma_start(ty_sb[:],
                              ty_dram[:].rearrange("j k m p f -> p (j k m) f"))
            nc.sync.dma_start(tf_sb[:],
                              tf_dram[:].rearrange("j k m p f -> p (j k m) f"))
            nc.sync.dma_start(kj_sb[:],
                              kj_dram[:].rearrange("j k p f -> p (j k) f"))
            nc.sync.dma_start(rho_sb[:],
                              rho_dram[:].rearrange("j p f -> p j f"))
            nc.sync.dma_start(ones_sb[:], ones_dram[:])
            nc.vector.memset(magic_sb[:], MAGIC)

            def load_small_consts():
                nc.sync.dma_start(beta1_sb[:], beta1_dram[:])
                nc.sync.dma_start(c3bda_sb[:], c3bda_dram[:])
                nc.sync.dma_start(c3bdb_sb[:], c3bdb_dram[:])
                nc.sync.dma_start(wbda_sb[:], wbda_dram[:])
                nc.sync.dma_start(wbdb_sb[:], wbdb_dram[:])
                nc.sync.dma_start(bd16_sb[:], bd16_dram[:])
                nc.sync.dma_start(sc_sb[:], sc_dram[:])

            def rsqrt(v_ap, k, tagp):
                """v_ap: [128,k] f32 SBUF (contiguous), >0. Returns [128,k] f32 tile."""
                sh = tiny.tile([128, k], I32, tag=f"rs_sh{tagp}")
                nc.vector.tensor_scalar(
                    out=sh[:], in0=v_ap.bitcast(I32), scalar1=1, scalar2=None,
                    op0=ALU.logical_shift_right)
                x = tiny.tile([128, k], F32, tag=f"rs_x{tagp}")
                nc.vector.tensor_tensor(
                    out=x[:].bitcast(I32), in0=magic_sb[:, :k], in1=sh[:],
                    op=ALU.subtract)
                xx = tiny.tile([128, k], F32, tag=f"rs_xx{tagp}")
                t3 = tiny.tile([128, k], F32, tag=f"rs_t3{tagp}")
                for _ in range(2):
                    nc.vector.tensor_tensor(out=xx[:], in0=x[:], in1=x[:], op=ALU.mult)
                    nc.vector.tensor_tensor(out=xx[:], in0=xx[:], in1=v_ap, op=ALU.mult)
                    nc.vector.tensor_scalar(out=t3[:], in0=xx[:], scalar1=-0.5,
                                            scalar2=1.5, op0=ALU.mult, op1=ALU.add)
                    nc.vector.tensor_tensor(out=x[:], in0=x[:], in1=t3[:], op=ALU.mult)
                return x

            for g in range(NG):
                smp = [g * G + s for s in range(G)]
                # ---- stage + transpose loads into pair tiles [128, 2A]
                in1 = [data.tile([128, 2 * A], BF16, tag="in1", name=f"in1_{s}")
                       for s in range(G)]
                for s in range(G):
                    nc.gpsimd.dma_start(stage[smp[s]], s_flat[smp[s]])
                for s in range(G):
                    for pt in range(2):
                        nc.sync.dma_start(
                            in1[s][:, A * pt : A * pt + A],
                            stage[smp[s], :, 128 * pt : 128 * pt + 128],
                            transpose=True)
                if g == 0:
                    load_small_consts()

                cur = in1
                xs = []
                c2ps = [c2psum.tile([128, A], F32, tag="c2ps", name=f"c2ps{bb}")
                        for bb in range(2)]
                # gelu accum sums feed the LN12 mean: [128, j, s, half]
                gxall = grp.tile([128, 3, G, 2], F32, tag="gxall")
                for j in range(3):
                    xj = [xdata.tile([128, 2 * A], BF16, tag=f"x{j}", name=f"x{j}_{s}")
                          for s in range(G)]
                    nxt = ([data.tile([128, 2 * A], BF16, tag=f"in{j+2}",
                                      name=f"in{j+2}_{s}", bufs=8) for s in range(G)]
                           if j < 2 else None)
                    for h in range(2):
                        # ---- phase 1 (half-group): y convs + per-sample stats
                        gy = grp.tile([128, 8], F32, tag="gy", name=f"gy{h}", bufs=2)
                        for sl in range(4):
                            s = 4 * h + sl
                            yps = [ypsum.tile([128, A], F32, tag="ypsum",
                                              name=f"yps{s}_{mm}") for mm in range(2)]
                            for mm in range(2):
                                for kk in range(2):
                                    nc.tensor.matmul(
                                        yps[mm][:], ty_sb[:, j, kk, mm, :],
                                        cur[s][:, A * kk : A * kk + A],
                                        start=(kk == 0), stop=(kk == 1))
                            st6 = stats.tile([128, 2, 6], F32, tag="st6")
                            for mm in range(2):
                                nc.vector.bn_stats(out=st6[:, mm, :], in_=yps[mm][:])
                            # gy[:,2sl]=mean_p, gy[:,2sl+1]=var_p -> then += mean^2
                            nc.vector.bn_aggr(out=gy[:, 2 * sl : 2 * sl + 2], in_=st6[:])
                            mm2 = stats.tile([128, 1], F32, tag="mm2")
                            nc.vector.tensor_tensor(
                                out=mm2[:], in0=gy[:, 2 * sl : 2 * sl + 1],
                                in1=gy[:, 2 * sl : 2 * sl + 1], op=ALU.mult)
                            nc.vector.tensor_tensor(
                                out=gy[:, 2 * sl + 1 : 2 * sl + 2], in0=mm2[:],
                                in1=gy[:, 2 * sl + 1 : 2 * sl + 2], op=ALU.add)
                        # ---- half-group aggregation -> gelu scale/bias
                        bcy = spsum.tile([128, 8], F32, tag="small", name=f"bcy{h}")
                        nc.tensor.matmul(bcy[:], ones_sb[:], gy[:], start=True, stop=True)
                        meanY = tiny.tile([128, 4], F32, tag=f"meanY{h}")
                        nc.vector.tensor_scalar_mul(meanY[:], bcy[:, 0:8:2], 1.0 / 128)
                        e2m = tiny.tile([128, 4], F32, tag=f"e2m{h}")
                        nc.vector.tensor_scalar_mul(e2m[:], bcy[:, 1:8:2], 1.0 / 128)
                        mm2b = tiny.tile([128, 4], F32, tag=f"mm2b{h}")
                        nc.vector.tensor_tensor(out=mm2b[:], in0=meanY[:], in1=meanY[:],
                                                op=ALU.mult)
                        varY = tiny.tile([128, 4], F32, tag=f"varY{h}")
                        nc.vector.tensor_tensor(out=varY[:], in0=e2m[:], in1=mm2b[:],
                                                op=ALU.subtract)
                        nc.vector.tensor_scalar_add(varY[:], varY[:], EPS)
                        rstdY = rsqrt(varY[:], 4, f"y{h}")
                        biasY = tiny.tile([128, 4], F32, tag=f"biasY{h}")
                        nc.vector.tensor_tensor(out=biasY[:], in0=meanY[:], in1=rstdY[:],
                                                op=ALU.mult)
                        nc.vector.scalar_tensor_tensor(
                            out=biasY[:], in0=biasY[:], scalar=-1.0,
                            in1=rho_sb[:, j, :], op0=ALU.mult, op1=ALU.mult)

                        # ---- phase 2 (half-group): z convs + gelu + residual add
                        for sl in range(4):
                            s = 4 * h + sl
                            zps = [zpsum.tile([128, A], F32, tag="zpsum",
                                              name=f"zps{s}_{mm}") for mm in range(2)]
                            for mm in range(2):
                                for kk in range(2):
                                    nc.tensor.matmul(
                                        zps[mm][:], tf_sb[:, j, kk, mm, :],
                                        cur[s][:, A * kk : A * kk + A],
                                        start=(kk == 0), stop=(kk == 1))
                            for mm in range(2):
                                nc.scalar.activation(
                                    out=xj[s][:, A * mm : A * mm + A], in_=zps[mm][:],
                                    func=ACTF.Gelu,
                                    bias=biasY[:, sl : sl + 1],
                                    scale=rstdY[:, sl : sl + 1],
                                    accum_out=gxall[:, j, s, mm : mm + 1])
                            if j < 2:
                                # residual add; last sample of each half-group
                                # on Pool (its 2.1us add never gates the next
                                # block's start), rest on DVE
                                if sl < 3:
                                    nc.vector.tensor_tensor(
                                        out=nxt[s][:], in0=cur[s][:], in1=xj[s][:],
                                        op=ALU.add)
                                else:
                                    nc.gpsimd.tensor_tensor(
                                        out=nxt[s][:], in0=cur[s][:], in1=xj[s][:],
                                        op=ALU.add)
                    xs.append(xj)
                    if j < 2:
                        cur = nxt

                # ---- c2 conv at group end (bank A = samples 0..3 at rows
                # 32sp..+15, bank B = 4..7); rows 16..31 of each slot:
                # row 16 = ones-col sums, rows 17..31 zero-filled
                for j in range(3):
                    for bank in range(2):
                        for sp in range(4):
                            s = 4 * bank + sp
                            for kk in range(2):
                                nc.tensor.matmul(
                                    c2ps[bank][32 * sp : 32 * sp + 32, :],
                                    kj_sb[:, j, kk, :],
                                    xs[j][s][:, A * kk : A * kk + A],
                                    start=(j == 0 and kk == 0),
                                    stop=(j == 2 and kk == 1),
                                    tile_position=(0, 32 * sp))

                # ---- LN12 mean per sample (variance dropped: eps-level term)
                t12 = tiny.tile([128, G, 2], F32, tag="t12")
                nc.vector.tensor_tensor(out=t12[:], in0=gxall[:, 0], in1=gxall[:, 1],
                                        op=ALU.add)
                nc.vector.tensor_tensor(out=t12[:], in0=t12[:], in1=gxall[:, 2],
                                        op=ALU.add)
                gsum = tiny.tile([128, G], F32, tag="gsum")
                nc.vector.tensor_tensor(out=gsum[:], in0=t12[:, :, 0],
                                        in1=t12[:, :, 1], op=ALU.add)
                bcm = spsum.tile([128, G], F32, tag="small", name="bcm")
                nc.tensor.matmul(bcm[:], ones_sb[:], gsum[:], start=True, stop=True)
                m12 = tiny.tile([128, G], F32, tag="m12")
                nc.vector.tensor_scalar_mul(m12[:], bcm[:], 1.0 / N12)
                # row-aligned [128,2] (colA=bankA rows 32s', colB=bankB)
                m12r = tiny.tile([128, 2], F32, tag="m12r")
                for sp in range(4):
                    rr = slice(32 * sp, 32 * sp + 16)
                    nc.gpsimd.tensor_copy(m12r[rr, 0:1], m12[rr, sp : sp + 1])
                    nc.gpsimd.tensor_copy(m12r[rr, 1:2], m12[rr, 4 + sp : 5 + sp])

                # LN16 combine inputs: per bank rhs [128,3]: mean, var+mean^2, beta*mean
                bcs = spsum.tile([128, 6], F32, tag="small", name="bcs")
                for bank in range(2):
                    mvc = stats.tile([128, 2], F32, tag="mvc")
                    stc = stats.tile([128, 1, 6], F32, tag="stc")
                    nc.vector.bn_stats(out=stc[:, 0, :], in_=c2ps[bank][:])
                    nc.vector.bn_aggr(out=mvc[:], in_=stc[:])
                    rhsc = stats.tile([128, 3], F32, tag="rhsc")
                    nc.vector.tensor_copy(rhsc[:, 0:1], mvc[:, 0:1])
                    mm2e = stats.tile([128, 1], F32, tag="mm2e")
                    nc.vector.tensor_tensor(out=mm2e[:], in0=mvc[:, 0:1],
                                            in1=mvc[:, 0:1], op=ALU.mult)
                    nc.vector.tensor_tensor(out=rhsc[:, 1:2], in0=mm2e[:],
                                            in1=mvc[:, 1:2], op=ALU.add)
                    nc.vector.tensor_tensor(out=rhsc[:, 2:3], in0=mvc[:, 0:1],
                                            in1=beta1_sb[:], op=ALU.mult)
                    nc.tensor.matmul(bcs[:, 3 * bank : 3 * bank + 3], bd16_sb[:],
                                     rhsc[:], start=True, stop=True)
                # var16 pipeline on [128,2] (col = bank); S0,S1,S2 at strided cols
                ex = tiny.tile([128, 2], F32, tag="ex")
                nc.vector.tensor_scalar_mul(ex[:], bcs[:, 0:6:3], 1.0 / 16)
                ex2 = tiny.tile([128, 2], F32, tag="ex2")
                nc.vector.tensor_scalar_mul(ex2[:], bcs[:, 1:6:3], 1.0 / 16)
                exa = tiny.tile([128, 2], F32, tag="exa")
                nc.vector.tensor_tensor(out=exa[:], in0=m12r[:], in1=bcs[:, 2:6:3],
                                        op=ALU.mult)
                nc.vector.tensor_scalar_mul(exa[:], exa[:], 1.0 / 16)
                ea = tiny.tile([128, 2], F32, tag="ea")
                nc.vector.tensor_scalar(out=ea[:], in0=m12r[:],
                                        scalar1=sc_sb[:, 0:1], scalar2=None,
                                        op0=ALU.mult)
                ea2 = tiny.tile([128, 2], F32, tag="ea2")
                nc.vector.tensor_tensor(out=ea2[:], in0=m12r[:], in1=m12r[:],
                                        op=ALU.mult)
                nc.vector.tensor_scalar(out=ea2[:], in0=ea2[:],
                                        scalar1=sc_sb[:, 1:2], scalar2=None,
                                        op0=ALU.mult)
                ctr = tiny.tile([128, 2], F32, tag="ctr")
                nc.vector.tensor_tensor(out=ctr[:], in0=ex[:], in1=ea[:],
                                        op=ALU.subtract)
                v16 = tiny.tile([128, 2], F32, tag="v16")
                nc.vector.scalar_tensor_tensor(out=v16[:], in0=exa[:], scalar=-2.0,
                                               in1=ex2[:], op0=ALU.mult, op1=ALU.add)
                nc.vector.tensor_tensor(out=v16[:], in0=v16[:], in1=ea2[:], op=ALU.add)
                nc.vector.tensor_tensor(out=ctr[:], in0=ctr[:], in1=ctr[:], op=ALU.mult)
                nc.vector.tensor_tensor(out=v16[:], in0=v16[:], in1=ctr[:],
                                        op=ALU.subtract)
                nc.vector.tensor_scalar_add(v16[:], v16[:], EPS)
                rstd16 = rsqrt(v16[:], 2, "c")

                # ---- scaled copy, q projection, final LN
                wtile = grp.tile([8, A], BF16, tag="wtile")
                nc.gpsimd.dma_start(wtile[:], w_dram[g * G : g * G + 8, :])
                q2 = stats.tile([128, 4], F32, tag="q2")
                outsb = [outp.tile([128, A], F32, tag="outsb", name=f"outsb{bb}")
                         for bb in range(2)]
                for bank in range(2):
                    c2sb = grp.tile([128, A], BF16, tag="c2sb")
                    nc.scalar.activation(out=c2sb[:], in_=c2ps[bank][:],
                                         func=ACTF.Copy,
                                         scale=rstd16[:, bank : bank + 1])
                    qps = spsum.tile([128, A], F32, tag="small", name=f"qps{bank}")
                    nc.tensor.matmul(qps[:], c3bda_sb[:] if bank == 0 else c3bdb_sb[:],
                                     c2sb[:], start=True, stop=False)
                    nc.tensor.matmul(qps[:], wbda_sb[:] if bank == 0 else wbdb_sb[:],
                                     wtile[:], start=False, stop=True)
                    stq = stats.tile([128, 1, 6], F32, tag="stq")
                    nc.vector.bn_stats(out=stq[:, 0, :], in_=qps[:])
                    nc.vector.bn_aggr(out=q2[:, 2 * bank : 2 * bank + 2], in_=stq[:])
                    va = tiny.tile([128, 1], F32, tag="va")
                    nc.vector.tensor_scalar_add(va[:], q2[:, 2 * bank + 1 : 2 * bank + 2], EPS)
                    rstda = rsqrt(va[:], 1, "a")
                    # out = (qps - mean)*rstd on ACT: Identity(qps*rstd + (-mean*rstd))
                    nbias = tiny.tile([128, 1], F32, tag="nbias")
                    nc.vector.tensor_tensor(out=nbias[:],
                                            in0=q2[:, 2 * bank : 2 * bank + 1],
                                            in1=rstda[:, 0:1], op=ALU.mult)
                    nc.vector.tensor_scalar_mul(nbias[:], nbias[:], -1.0)
                    nc.scalar.activation(out=outsb[bank][:], in_=qps[:],
                                         func=ACTF.Identity,
                                         bias=nbias[:, 0:1], scale=rstda[:, 0:1])
                # valid rows: bank A at 32*sp, bank B at 32*sp+16 -> out[g*8+...]
                for bank in range(2):
                    src = outsb[bank][:].rearrange("(sp u) a -> sp u a", u=32)
                    nc.scalar.dma_start(
                        out_dram[g * G + 4 * bank : g * G + 4 * bank + 4, :],
                        src[:, 16 * bank, :])
    nc.compile()
    return nc


_CACHE = {}


def kernel(**inputs):
    inputs = {k: np.asarray(v) for k, v in inputs.items()}
    consts = build_consts(inputs)
    if "nc" not in _CACHE:
        _CACHE["nc"] = build_program()
    nc = _CACHE["nc"]

    sc = np.zeros((128, 2), np.float32)
    sc[:, 0] = consts["meanbeta"]
    sc[:, 1] = consts["ebeta2"]
    base = {
        "ty": np.ascontiguousarray(consts["ty"]),
        "tf": np.ascontiguousarray(consts["tf"]),
        "kj": np.ascontiguousarray(consts["kj"]),
        "rho": np.ascontiguousarray(consts["rho"]),
        "beta1": np.ascontiguousarray(consts["beta1"]),
        "c3bda": np.ascontiguousarray(consts["c3bda"]),
        "c3bdb": np.ascontiguousarray(consts["c3bdb"]),
        "wbda": np.ascontiguousarray(consts["wbda"]),
        "wbdb": np.ascontiguousarray(consts["wbdb"]),
        "bd16": np.ascontiguousarray(consts["bd16"]),
        "ones": np.ascontiguousarray(consts["ones"]),
        "sc": sc,
    }
    in_maps = []
    for c in range(NCORES):
        m = dict(base)
        m["s"] = np.ascontiguousarray(inputs["s"][c * SPC : (c + 1) * SPC])
        m["w"] = np.ascontiguousarray(inputs["w"][c * SPC : (c + 1) * SPC])
        in_maps.append(m)
    _CACHE["in_maps"] = in_maps
    res = run_bass_kernel_spmd(nc, in_maps, core_ids=list(range(NCORES)))
    out = np.concatenate([r["out"] for r in res.results], axis=0)
    return out.astype(np.float32)
